# revision 7
# baseline (speedup 1.0000x reference)
import numpy as np

B = 8
SEQ = 4096
D = 1024
N_BASE = 10000.0
N_CORES = 8
SPC = SEQ // N_CORES   # 512 seq rows per core
JT = SPC // 128        # 4 rows per partition
BUFS = 5

# HWDGE splits a dma_start's descriptors equally over the largest divisor of
# the descriptor count <= 16 engines, sequential blocks from engine 0.
# Engine 15 is a frequent cross-core-contention straggler (~0.81x speed), so
# chunks 6-7 load only partitions [0:120] (120 descs -> engines 0-14, engine
# 15 idle) and their partitions [120:128] ride a small overflow block.
# Engine 15 then carries 6/8 = 0.75x of the bulk traffic; engines 0-14 keep
# v2-identical 16 KiB descriptors and byte counts.
N_FULL = 6                             # chunks with all 128 partitions
P_CUT = 120                            # partition cut for chunks 6-7
N_OVF = (B - N_FULL) * (128 - P_CUT) * JT   # 64 rows
CH_FULL_ROWS = 128 * JT                # 512
CH_CUT_ROWS = P_CUT * JT               # 480
CUT_BASE = N_FULL * CH_FULL_ROWS       # 3072
OVF_BASE = CUT_BASE + (B - N_FULL) * CH_CUT_ROWS  # 4032
N_ROWS = OVF_BASE + N_OVF              # 4096

_CACHE = {}


def _compute_pe() -> np.ndarray:
    """Mirror of the reference _pos_encoding (default jax backend, f32)."""
    import jax
    import jax.numpy as jnp

    pos = jnp.arange(SEQ, dtype=jnp.float32)[:, None]
    i = jnp.arange(D // 2, dtype=jnp.float32)
    denom = jnp.power(jnp.float32(N_BASE), 2.0 * i / jnp.float32(D))
    ang = pos / denom
    pe = jnp.stack([jnp.sin(ang), jnp.cos(ang)], axis=-1).reshape(SEQ, D)
    return np.asarray(jax.device_get(pe), dtype=np.float32)


def _ovf_seq_idx():
    """(batch, within-core-seq-row) per overflow row j = (s*JT+u)*(B-N_FULL)+(b-N_FULL)."""
    nb = B - N_FULL
    j = np.arange(N_OVF)
    b = N_FULL + (j % nb)
    r = P_CUT * JT + j // nb       # seq rows [480:512)
    return b, r


def _repack(x, c):
    xs = x[:, c * SPC : (c + 1) * SPC, :]  # [B, 512, D]
    out = np.empty((N_ROWS, D), dtype=np.float32)
    for g in range(N_FULL):
        out[g * CH_FULL_ROWS : (g + 1) * CH_FULL_ROWS] = xs[g]
    for g in range(N_FULL, B):
        base = CUT_BASE + (g - N_FULL) * CH_CUT_ROWS
        out[base : base + CH_CUT_ROWS] = xs[g][:CH_CUT_ROWS]
    ob, orow = _ovf_seq_idx()
    out[OVF_BASE:] = xs[ob, orow, :]
    return out


def _unpack(y, c, out):
    """Scatter core c's kernel output y [N_ROWS, D] into out [B, SEQ, D]."""
    dst = out[:, c * SPC : (c + 1) * SPC, :]
    for g in range(N_FULL):
        dst[g] = y[g * CH_FULL_ROWS : (g + 1) * CH_FULL_ROWS]
    for g in range(N_FULL, B):
        base = CUT_BASE + (g - N_FULL) * CH_CUT_ROWS
        dst[g][:CH_CUT_ROWS] = y[base : base + CH_CUT_ROWS]
    ob, orow = _ovf_seq_idx()
    dst[ob, orow, :] = y[OVF_BASE:]


def _pe_inputs(pe, c):
    pes = np.ascontiguousarray(pe[c * SPC : (c + 1) * SPC, :])
    ob, orow = _ovf_seq_idx()
    pe_ovf = np.ascontiguousarray(pes[orow])
    return pes, pe_ovf


def _build_program(bufs=BUFS):
    import concourse.bacc as bacc
    import concourse.mybir as mybir
    import concourse.tile as tile

    nc = bacc.Bacc("TRN2")
    f32 = mybir.dt.float32
    x_in = nc.declare_dram_parameter("x", [N_ROWS, D], f32, isOutput=False)
    pe_in = nc.declare_dram_parameter("pe", [SPC, D], f32, isOutput=False)
    po_in = nc.declare_dram_parameter("pe_ovf", [N_OVF, D], f32, isOutput=False)
    y_out = nc.declare_dram_parameter("y", [N_ROWS, D], f32, isOutput=True)

    with tile.TileContext(nc) as tc:
        with (
            tc.tile_pool(name="pe_pool", bufs=1) as pe_pool,
            tc.tile_pool(name="x_pool", bufs=bufs) as x_pool,
            tc.tile_pool(name="ovf_pool", bufs=1) as ovf_pool,
        ):
            pe_t = pe_pool.tile([128, JT, D], f32)
            nc.sync.dma_start(
                out=pe_t[:], in_=pe_in.rearrange("(p u) d -> p u d", u=JT)
            )
            po_t = ovf_pool.tile([N_OVF, 1, D], f32, tag="po")
            nc.sync.dma_start(
                out=po_t[:], in_=po_in.rearrange("(p u) d -> p u d", u=1)
            )
            ov_t = ovf_pool.tile([N_OVF, 1, D], f32, tag="ov")
            nc.sync.dma_start(
                out=ov_t[:],
                in_=x_in[OVF_BASE:, :].rearrange("(p u) d -> p u d", u=1),
            )
            nc.vector.tensor_add(ov_t[:], ov_t[:], po_t[:])
            nc.scalar.dma_start(
                out=y_out[OVF_BASE:, :].rearrange("(p u) d -> p u d", u=1),
                in_=ov_t[:],
            )
            for g in range(B):
                xt = x_pool.tile([128, JT, D], f32)
                if g < N_FULL:
                    rows = slice(g * CH_FULL_ROWS, (g + 1) * CH_FULL_ROWS)
                    sl = xt[:]
                    pe_sl = pe_t[:]
                else:
                    base = CUT_BASE + (g - N_FULL) * CH_CUT_ROWS
                    rows = slice(base, base + CH_CUT_ROWS)
                    sl = xt[0:P_CUT, :, :]
                    pe_sl = pe_t[0:P_CUT, :, :]
                nc.sync.dma_start(
                    out=sl,
                    in_=x_in[rows, :].rearrange("(p k) d -> p k d", k=JT),
                )
                nc.vector.tensor_add(sl, sl, pe_sl)
                nc.scalar.dma_start(
                    out=y_out[rows, :].rearrange("(p k) d -> p k d", k=JT),
                    in_=sl,
                )
    if not nc.is_finalized():
        nc.finalize()
    return nc


def _get_state():
    if "nc" not in _CACHE:
        _CACHE["nc"] = _build_program()
    if "pe" not in _CACHE:
        _CACHE["pe"] = _compute_pe()
    return _CACHE["nc"], _CACHE["pe"]


def _make_in_maps(x):
    nc, pe = _get_state()
    in_maps = []
    for c in range(N_CORES):
        pes, pe_ovf = _pe_inputs(pe, c)
        in_maps.append({"x": _repack(x, c), "pe": pes, "pe_ovf": pe_ovf})
    return nc, in_maps


def kernel(x, seq_len=None, **_):
    from concourse.bass_utils import run_bass_kernel_spmd

    x = np.asarray(x, dtype=np.float32)
    assert x.shape == (B, SEQ, D)
    if seq_len is not None:
        assert int(np.asarray(seq_len)) == SEQ

    nc, in_maps = _make_in_maps(x)
    res = run_bass_kernel_spmd(nc, in_maps, list(range(N_CORES))).results

    out = np.empty((B, SEQ, D), dtype=np.float32)
    for c in range(N_CORES):
        _unpack(res[c]["y"], c, out)
    return out


# revision 9
# speedup vs baseline: 1.0312x; 1.0312x over previous
import numpy as np

B = 8
SEQ = 4096
D = 1024
N_BASE = 10000.0
N_CORES = 8
SPC = SEQ // N_CORES  # seq rows per core
JT = SPC // 128       # 128-row chunks per core
G_DEFAULT = 1         # batches per DMA chunk (chunk = G*2MiB)
BUFS = 5

_CACHE = {}


def _compute_pe() -> np.ndarray:
    """Mirror of the reference _pos_encoding (default jax backend, f32)."""
    import jax
    import jax.numpy as jnp

    pos = jnp.arange(SEQ, dtype=jnp.float32)[:, None]
    i = jnp.arange(D // 2, dtype=jnp.float32)
    denom = jnp.power(jnp.float32(N_BASE), 2.0 * i / jnp.float32(D))
    ang = pos / denom
    pe = jnp.stack([jnp.sin(ang), jnp.cos(ang)], axis=-1).reshape(SEQ, D)
    return np.asarray(jax.device_get(pe), dtype=np.float32)


def _repack(x, c, G):
    xs = np.ascontiguousarray(x[:, c * SPC : (c + 1) * SPC, :])
    NG = B // G
    return np.ascontiguousarray(
        xs.reshape(NG, G, 128, JT, D).transpose(0, 2, 1, 3, 4)
    ).reshape(B * SPC, D)


def _unpack(y, G):
    NG = B // G
    return np.ascontiguousarray(
        y.reshape(NG, 128, G, JT, D).transpose(0, 2, 1, 3, 4)
    ).reshape(B, SPC, D)


def _build_program(G, bufs=BUFS):
    import concourse.bacc as bacc
    import concourse.mybir as mybir
    import concourse.tile as tile

    NG = B // G
    K = G * JT
    nc = bacc.Bacc("TRN2")
    f32 = mybir.dt.float32
    x_in = nc.declare_dram_parameter("x", [B * SPC, D], f32, isOutput=False)
    pe_in = nc.declare_dram_parameter("pe", [SPC, D], f32, isOutput=False)
    y_out = nc.declare_dram_parameter("y", [B * SPC, D], f32, isOutput=True)

    with tile.TileContext(nc) as tc:
        with (
            tc.tile_pool(name="pe_pool", bufs=1) as pe_pool,
            tc.tile_pool(name="x_pool", bufs=min(bufs, NG)) as x_pool,
        ):
            pe_t = pe_pool.tile([128, JT, D], f32)
            pe_ap = pe_in.rearrange("(p u) d -> p u d", u=JT)
            nc.sync.dma_start(out=pe_t[:], in_=pe_ap)
            for g in range(NG):
                xs = x_in[g * 128 * K : (g + 1) * 128 * K, :].rearrange(
                    "(p k) d -> p k d", k=K
                )
                xt = x_pool.tile([128, K, D], f32)
                nc.sync.dma_start(out=xt[:], in_=xs)
                for bb in range(G):
                    sl = xt[:, bb * JT : (bb + 1) * JT, :]
                    nc.vector.tensor_add(sl, sl, pe_t[:])
                ys = y_out[g * 128 * K : (g + 1) * 128 * K, :].rearrange(
                    "(p k) d -> p k d", k=K
                )
                nc.scalar.dma_start(out=ys, in_=xt[:])
    if not nc.is_finalized():
        nc.finalize()
    return nc


def _get_state(G=G_DEFAULT):
    if G not in _CACHE:
        _CACHE[G] = _build_program(G)
    if "pe" not in _CACHE:
        _CACHE["pe"] = _compute_pe()
    return _CACHE[G], _CACHE["pe"]


def kernel(x, seq_len=None, **_):
    from concourse.bass_utils import run_bass_kernel_spmd

    x = np.asarray(x, dtype=np.float32)
    assert x.shape == (B, SEQ, D)
    if seq_len is not None:
        assert int(np.asarray(seq_len)) == SEQ

    G = G_DEFAULT
    nc, pe = _get_state(G)
    in_maps = []
    for c in range(N_CORES):
        pes = np.ascontiguousarray(pe[c * SPC : (c + 1) * SPC, :])
        in_maps.append({"x": _repack(x, c, G), "pe": pes})

    res = run_bass_kernel_spmd(nc, in_maps, list(range(N_CORES))).results

    out = np.empty((B, SEQ, D), dtype=np.float32)
    for c in range(N_CORES):
        out[:, c * SPC : (c + 1) * SPC, :] = _unpack(res[c]["y"], G)
    return out
